# revision 2
# baseline (speedup 1.0000x reference)
import sys

sys.path.insert(0, "/opt/trn_rl_repo")
import numpy as np
import ml_dtypes

BF16 = ml_dtypes.bfloat16
S, B, H, DK, DM = 2048, 2, 16, 64, 1024
HPC = 4            # heads per core
EPC = HPC * DK     # 256 embed dims per core
VW = HPC * (DK + 1)  # 260: 4 heads x (64 dims + rowsum column)
NEG = -1e9

_prog = None


def _build():
    import concourse.tile as tile
    from concourse import bacc, mybir

    f32 = mybir.dt.float32
    bf16 = mybir.dt.bfloat16
    Exp = mybir.ActivationFunctionType.Exp

    nc = bacc.Bacc("TRN2", target_bir_lowering=False, debug=False)
    xq_d = nc.declare_dram_parameter("xq", [DM, S], bf16, isOutput=False)
    xk_d = nc.declare_dram_parameter("xk", [DM, S], bf16, isOutput=False)
    xv_d = nc.declare_dram_parameter("xv", [DM, S], bf16, isOutput=False)
    wq_d = nc.declare_dram_parameter("wq", [DM, EPC], bf16, isOutput=False)
    wk_d = nc.declare_dram_parameter("wk", [DM, EPC], bf16, isOutput=False)
    wv_d = nc.declare_dram_parameter("wv", [DM, VW], bf16, isOutput=False)
    bq_d = nc.declare_dram_parameter("bq", [1, EPC], bf16, isOutput=False)
    bk_d = nc.declare_dram_parameter("bk", [1, EPC], bf16, isOutput=False)
    bv_d = nc.declare_dram_parameter("bv", [1, VW], bf16, isOutput=False)
    wo_d = nc.declare_dram_parameter("wo", [EPC, DM], bf16, isOutput=False)
    cst_d = nc.declare_dram_parameter("cst", [128, 256], bf16, isOutput=False)
    out_d = nc.declare_dram_parameter("outT", [DM, S], f32, isOutput=True)

    with tile.TileContext(nc) as tc:
        with tc.tile_pool(name="sb", bufs=1) as sb:
            cst_sb = sb.tile([128, 256], bf16)
            nc.sync.dma_start(cst_sb[:], cst_d[:])
            ident = cst_sb[:, 0:128]
            tri = cst_sb[:, 128:256]
            ones = sb.tile([1, 512], bf16)
            nc.vector.memset(ones[:], 1.0)

            wq_sb, wk_sb, wv_sb, wo_sb, xv_sb = [], [], [], [], []
            for dt in range(8):
                t = sb.tile([128, EPC], bf16, name=f"wq{dt}")
                nc.sync.dma_start(t[:], wq_d[dt * 128:(dt + 1) * 128, :])
                wq_sb.append(t)
            bq_sb = sb.tile([1, EPC], bf16)
            nc.sync.dma_start(bq_sb[:], bq_d[:])
            for dt in range(8):
                t = sb.tile([128, EPC], bf16, name=f"wk{dt}")
                nc.sync.dma_start(t[:], wk_d[dt * 128:(dt + 1) * 128, :])
                wk_sb.append(t)
            bk_sb = sb.tile([1, EPC], bf16)
            nc.sync.dma_start(bk_sb[:], bk_d[:])
            for dt in range(8):
                t = sb.tile([128, VW], bf16, name=f"wv{dt}")
                nc.sync.dma_start(t[:], wv_d[dt * 128:(dt + 1) * 128, :])
                wv_sb.append(t)
            bv_sb = sb.tile([1, VW], bf16)
            nc.sync.dma_start(bv_sb[:], bv_d[:])
            for et in range(2):
                t = sb.tile([128, DM], bf16, name=f"wo{et}")
                nc.sync.dma_start(t[:], wo_d[et * 128:(et + 1) * 128, :])
                wo_sb.append(t)
            for dt in range(8):
                t = sb.tile([128, S], bf16, name=f"xv{dt}")
                nc.sync.dma_start(t[:], xv_d[dt * 128:(dt + 1) * 128, :])
                xv_sb.append(t)

            Qt_sb = [sb.tile([128, S], bf16, name=f"Qt{et}") for et in range(2)]
            Kt_sb = [sb.tile([128, S], bf16, name=f"Kt{et}") for et in range(2)]
            ctx_sb = [sb.tile([128, 16 * DK], bf16, name=f"ctx{h}") for h in range(4)]
            ctxT_sb = [sb.tile([128, S], bf16, name=f"ctxT{et}") for et in range(2)]
            V_sb = []

            # ---- phase A: projections ----
            with tc.tile_pool(name="psA", bufs=1, space="PSUM") as psA:

                def proj(x_d, w_sb, b_sb, out_sb):
                    ps = [psA.tile([128, 512], f32, name="ps_qk", tag="pA", bufs=8)
                          for _ in range(8)]
                    for dt in range(8):
                        xt = sb.tile([128, S], bf16, name="xin", tag="xin", bufs=3)
                        nc.gpsimd.dma_start(xt[:], x_d[dt * 128:(dt + 1) * 128, :])
                        for et in range(2):
                            for qcc in range(4):
                                nc.tensor.matmul(
                                    ps[et * 4 + qcc][:],
                                    w_sb[dt][:, et * 128:(et + 1) * 128],
                                    xt[:, qcc * 512:(qcc + 1) * 512],
                                    start=(dt == 0), stop=False)
                    for et in range(2):
                        for qcc in range(4):
                            p = ps[et * 4 + qcc]
                            nc.tensor.matmul(p[:], b_sb[0:1, et * 128:(et + 1) * 128],
                                             ones[0:1, 0:512], start=False, stop=True)
                            nc.vector.tensor_copy(
                                out_sb[et][:, qcc * 512:(qcc + 1) * 512], p[:])

                proj(xq_d, wq_sb, bq_sb, Qt_sb)
                proj(xk_d, wk_sb, bk_sb, Kt_sb)
                for kt in range(16):
                    pv = psA.tile([128, VW], f32, name="ps_v", tag="pA", bufs=8)
                    for dt in range(8):
                        nc.tensor.matmul(pv[:], xv_sb[dt][:, kt * 128:(kt + 1) * 128],
                                         wv_sb[dt][:], start=(dt == 0), stop=False)
                    nc.tensor.matmul(pv[:], ones[0:1, 0:128], bv_sb[0:1, :],
                                     start=False, stop=True)
                    vt = sb.tile([128, VW], bf16, name=f"v{kt}")
                    nc.vector.tensor_copy(vt[:], pv[:])
                    V_sb.append(vt)

            # ---- phase B: attention ----
            with tc.tile_pool(name="psB", bufs=1, space="PSUM") as psB:
                for qc in range(4):
                    for pair in range(2):
                        cps = [psB.tile([128, VW], f32, name=f"ps_ctx{h}",
                                        tag="ctx", bufs=2) for h in range(2)]
                        for kt in range(4 * qc + 4):
                            diag = kt - 4 * qc
                            span = psB.tile([128, 1024], f32, name="ps_span",
                                            tag="span", bufs=3)
                            for h in range(2):
                                nc.tensor.matmul(
                                    span[:, h * 512:(h + 1) * 512],
                                    Kt_sb[pair][h * 64:(h + 1) * 64, kt * 128:(kt + 1) * 128],
                                    Qt_sb[pair][h * 64:(h + 1) * 64, qc * 512:(qc + 1) * 512],
                                    start=True, stop=(diag < 0), skip_group_check=True)
                            if diag >= 0:
                                for h in range(2):
                                    c0 = h * 512 + diag * 128
                                    nc.tensor.matmul(span[:, c0:c0 + 128], ident, tri,
                                                     start=False, stop=True,
                                                     skip_group_check=True)
                            pt = sb.tile([128, 1024], bf16, name="pt", tag="pt", bufs=3)
                            nc.scalar.activation(pt[:], span[:], Exp)
                            for h in range(2):
                                hh = pair * 2 + h
                                for j in range(4):
                                    if kt <= 4 * qc + j:
                                        nc.tensor.matmul(
                                            cps[h][:, j * 65:(j + 1) * 65],
                                            pt[:, h * 512 + j * 128:h * 512 + (j + 1) * 128],
                                            V_sb[kt][:, hh * 65:(hh + 1) * 65],
                                            start=(kt == 0 and j == 0),
                                            stop=(kt == 4 * qc + j),
                                            skip_group_check=True)
                        for h in range(2):
                            hh = pair * 2 + h
                            for j in range(4):
                                qt = qc * 4 + j
                                r = sb.tile([128, 1], f32, name="r", tag="r", bufs=4)
                                nc.vector.reciprocal(r[:], cps[h][:, j * 65 + 64:(j + 1) * 65])
                                nc.vector.tensor_scalar_mul(
                                    ctx_sb[hh][:, qt * 64:(qt + 1) * 64],
                                    cps[h][:, j * 65:j * 65 + 64], r[:, 0:1])

            # ---- phase C: transpose ctx to [e, q]; phase D: out projection ----
            with tc.tile_pool(name="psC", bufs=1, space="PSUM") as psC:
                for pair in range(2):
                    for qt in range(16):
                        ptr = psC.tile([128, 128], bf16, name="ps_tr", tag="tr", bufs=2)
                        for h in range(2):
                            hh = pair * 2 + h
                            nc.tensor.transpose(ptr[h * 64:(h + 1) * 64, :],
                                                ctx_sb[hh][:, qt * 64:(qt + 1) * 64],
                                                ident)
                        nc.vector.tensor_copy(ctxT_sb[pair][:, qt * 128:(qt + 1) * 128],
                                              ptr[:])
                qi = 0
                for qc in range(4):
                    for mt in range(8):
                        po = psC.tile([128, 512], f32, name="ps_out", tag="out", bufs=3)
                        for et in range(2):
                            nc.tensor.matmul(po[:],
                                             wo_sb[et][:, mt * 128:(mt + 1) * 128],
                                             ctxT_sb[et][:, qc * 512:(qc + 1) * 512],
                                             start=(et == 0), stop=(et == 1))
                        y = sb.tile([128, 512], f32, name="y", tag="y", bufs=3)
                        nc.vector.tensor_copy(y[:], po[:])
                        eng = nc.sync if qi % 2 == 0 else nc.gpsimd
                        eng.dma_start(out_d[mt * 128:(mt + 1) * 128,
                                            qc * 512:(qc + 1) * 512], y[:])
                        qi += 1

    nc.compile()
    return nc


def _make_cst():
    cst = np.zeros((128, 256), np.float32)
    cst[:, 0:128] = np.eye(128, dtype=np.float32)
    kk = np.arange(128)[:, None]
    qq = np.arange(128)[None, :]
    cst[:, 128:256] = np.where(kk > qq, np.float32(NEG), np.float32(0.0))
    return cst.astype(BF16)


def _prep_in_maps(query, key, value, Wq, bq, Wk, bk, Wv, bv, Wo):
    WqT = (Wq.T.astype(np.float32) * 0.125)
    WkT = Wk.T.astype(np.float32)
    WvT = Wv.T.astype(np.float32)
    WoT = Wo.T.astype(np.float32)
    bqs = bq.astype(np.float32) * 0.125
    cst = _make_cst()
    in_maps = []
    for c in range(8):
        b, g = c // 4, c % 4
        e0 = EPC * g
        wv_arr = np.zeros((DM, VW), np.float32)
        bv_arr = np.zeros((1, VW), np.float32)
        for j in range(HPC):
            wv_arr[:, 65 * j:65 * j + 64] = WvT[:, e0 + 64 * j:e0 + 64 * j + 64]
            bv_arr[0, 65 * j:65 * j + 64] = bv[e0 + 64 * j:e0 + 64 * j + 64]
            bv_arr[0, 65 * j + 64] = 1.0
        in_maps.append({
            "xq": np.ascontiguousarray(query[:, b, :].T).astype(BF16),
            "xk": np.ascontiguousarray(key[:, b, :].T).astype(BF16),
            "xv": np.ascontiguousarray(value[:, b, :].T).astype(BF16),
            "wq": np.ascontiguousarray(WqT[:, e0:e0 + EPC]).astype(BF16),
            "wk": np.ascontiguousarray(WkT[:, e0:e0 + EPC]).astype(BF16),
            "wv": wv_arr.astype(BF16),
            "bq": bqs[e0:e0 + EPC].reshape(1, EPC).astype(BF16),
            "bk": bk[e0:e0 + EPC].reshape(1, EPC).astype(BF16),
            "bv": bv_arr.astype(BF16),
            "wo": np.ascontiguousarray(WoT[e0:e0 + EPC, :]).astype(BF16),
            "cst": cst,
        })
    return in_maps


def _gather(results, bo):
    out = np.empty((S, B, DM), np.float32)
    for b in range(B):
        acc = np.zeros((DM, S), np.float32)
        for g in range(4):
            acc += results[4 * b + g]["outT"]
        acc += bo.astype(np.float32)[:, None]
        out[:, b, :] = acc.T
    return out


def _is_causal(mask):
    m = np.asarray(mask)
    if m.shape != (B, 1, S, S):
        return False
    neg = np.isneginf(m)
    causal = np.triu(np.ones((S, S), dtype=bool), k=1)
    return bool((neg == causal[None, None]).all())


def _numpy_ref(query, key, value, mask, Wq, bq, Wk, bk, Wv, bv, Wo, bo):
    q = (query @ Wq.T + bq).reshape(S, B, H, DK)
    k = (key @ Wk.T + bk).reshape(S, B, H, DK)
    v = (value @ Wv.T + bv).reshape(S, B, H, DK)
    scores = np.einsum("qbhd,kbhd->bhqk", q, k) / np.sqrt(DK)
    scores = np.where(np.isneginf(mask), np.float32(-1e9), scores)
    scores = scores - scores.max(axis=-1, keepdims=True)
    e = np.exp(scores)
    attn = e / e.sum(axis=-1, keepdims=True)
    ctx = np.einsum("bhqk,kbhd->qbhd", attn, v).reshape(S, B, DM)
    return (ctx @ Wo.T + bo).astype(np.float32)


def kernel(**inputs):
    global _prog
    ins = {k: np.asarray(v) for k, v in inputs.items()}
    if not _is_causal(ins["mask"]):
        return _numpy_ref(**ins)
    if _prog is None:
        _prog = _build()
    from concourse.bass_utils import run_bass_kernel_spmd

    in_maps = _prep_in_maps(ins["query"], ins["key"], ins["value"],
                            ins["Wq"], ins["bq"], ins["Wk"], ins["bk"],
                            ins["Wv"], ins["bv"], ins["Wo"])
    res = run_bass_kernel_spmd(_prog, in_maps, list(range(8)))
    return _gather(res.results, ins["bo"])
